# revision 6
# baseline (speedup 1.0000x reference)
"""Causal self-attention (LayerNorm + fused QKV + causal/len-masked softmax
attention + out-proj) on 8 Trainium2 NeuronCores, data-parallel over batch.

Contract: kernel(**inputs) takes the full unsharded inputs (B=8, T=1024,
D=1024, H=16) and returns the full (B, T, D) float32 output. Each core
processes one batch element; there are no cross-core collectives.

Device program per core (see build_attention):
  - LayerNorm stats from x^T via scalar-Square + ones-column matmuls into
    PSUM rows [1, T]; r = 1/sqrt(var+eps) and rmu = -r*mu stay in row form
    and are partition-broadcast on GpSimd (no DRAM scratch round-trips).
  - x is normalized in place (xn = x*r + rmu), so Q/K/V projections are
    pure matmuls; biases are fused into the PSUM evacuations.
  - Q^T/K^T in (j, t) layout, V in (t, j) layout with a ones column per
    head so att@V also produces the softmax denominator.
  - scores^T per head with K=64 row-packed matmul pairs (tile_position);
    exp on the scalar engine with the kv-length mask as per-partition
    bias; causal masking via gpsimd affine_select on diagonal blocks.
  - Softmax denominator: reciprocal of the PSUM l-row on vector, then
    GpSimd partition_broadcast, then one fused multiply during evacuation.
  - Out-proj emits out^T in bf16; the host casts/transposes back.
"""

import math
import sys

for _p in ('/opt/trn_rl_repo', '/opt/trn_rl_repo/pypackages', '/root/.axon_site'):
    if _p not in sys.path:
        sys.path.insert(0, _p)

import numpy as np
import ml_dtypes

import concourse.bass as bass
import concourse.mybir as mybir

dt = mybir.dt
F32 = dt.float32
BF16 = dt.bfloat16
Alu = mybir.AluOpType
Act = mybir.ActivationFunctionType

P = 128
B, T_FULL, D_FULL, H_FULL = 8, 1024, 1024, 16
NEG = -1e9
EPS = 1e-5


def build_attention(nc, tc, T=1024, D=1024, H=16, EPS=1e-5):
    hd = D // H
    assert hd == 64, "row-packed scores assume head_dim == 64"
    ND = D // P              # d-chunks (contraction for projections)
    NT = T // P              # t-chunks of 128 (s-chunks too)
    TF = 512                 # free-dim t chunk (psum bank limit for f32)
    NTF = T // TF
    NQ = D // P              # number of Q chunks (K chunks follow)
    VF = 512                 # j free-chunk width for V
    NVF = D // VF
    NPAIR = H // 2
    scale = 1.0 / math.sqrt(hd)

    # ---- DRAM parameters ----
    xT = nc.declare_dram_parameter("xT", [D, T], BF16, isOutput=False)
    wqk = nc.declare_dram_parameter("wqk", [D, 2 * D], BF16, isOutput=False)
    wv = nc.declare_dram_parameter("wv", [D, D], BF16, isOutput=False)
    wout = nc.declare_dram_parameter("wout", [D, D], BF16, isOutput=False)
    c2qk = nc.declare_dram_parameter("c2qk", [P, 2 * NQ], F32, isOutput=False)
    bv = nc.declare_dram_parameter("bv", [1, D], F32, isOutput=False)
    kvm = nc.declare_dram_parameter("kvm", [P, NT], F32, isOutput=False)
    bout = nc.declare_dram_parameter("bout", [P, ND], F32, isOutput=False)
    out = nc.declare_dram_parameter("out", [D, T], BF16, isOutput=True)

    import contextlib
    ctx = contextlib.ExitStack()
    singles = ctx.enter_context(tc.tile_pool(name="singles", bufs=1))

    # ---- standing SBUF tiles; DMA order: xT, wv, consts, wqk, wout ----
    # Pools are a strict stack: create in reverse-lifetime order
    # (wout > wqk > wv/rb), emit DMAs in arrival-priority order.
    wop = ctx.enter_context(tc.tile_pool(name="wout_p", bufs=1))
    wqk_ctx = contextlib.ExitStack()
    wqkp = wqk_ctx.enter_context(tc.tile_pool(name="wqk_p", bufs=1))
    wv_ctx = contextlib.ExitStack()
    wvp = wv_ctx.enter_context(tc.tile_pool(name="wv_p", bufs=1))

    xn_sb = []
    for dc in range(ND):
        t = singles.tile([P, T], BF16, name=f"xn{dc}", tag=f"xn{dc}")
        nc.sync.dma_start(out=t, in_=xT[dc * P:(dc + 1) * P, :])
        xn_sb.append(t)
    wv_sb = []
    for dc in range(ND):
        w = wvp.tile([P, D], BF16, name=f"wv{dc}", tag=f"wv{dc}")
        nc.sync.dma_start(out=w, in_=wv[dc * P:(dc + 1) * P, :])
        wv_sb.append(w)
    c2qk_sb = singles.tile([P, 2 * NQ], F32, tag="c2qk")
    nc.sync.dma_start(out=c2qk_sb, in_=c2qk[:, :])
    bv_row = singles.tile([1, D], F32, tag="bv_row")
    nc.sync.dma_start(out=bv_row, in_=bv[:, :])
    kvm_sb = singles.tile([P, NT], F32, tag="kvm")
    nc.sync.dma_start(out=kvm_sb, in_=kvm[:, :])
    bout_sb = singles.tile([P, ND], F32, tag="bout")
    nc.sync.dma_start(out=bout_sb, in_=bout[:, :])
    wqk_sb = []
    for dc in range(ND):
        w = wqkp.tile([P, 2 * D], BF16, name=f"wqk{dc}", tag=f"wqk{dc}")
        nc.sync.dma_start(out=w, in_=wqk[dc * P:(dc + 1) * P, :])
        wqk_sb.append(w)
    wout_sb = []
    for vc in range(ND):
        w = wop.tile([P, D], BF16, name=f"wout{vc}", tag=f"wout{vc}")
        nc.sync.dma_start(out=w, in_=wout[vc * P:(vc + 1) * P, :])
        wout_sb.append(w)

    ones1 = singles.tile([P, 1], BF16, tag="ones1")
    nc.vector.memset(ones1, 1.0)
    eps_t = singles.tile([1, 1], F32, tag="eps")
    nc.vector.memset(eps_t, EPS)
    # Load the sqrt act table before the squares so only one later switch
    # (to the exp table) happens.
    sqd = singles.tile([1, 1], F32, tag="sqd")
    nc.scalar.activation(out=sqd, in_=eps_t, func=Act.Sqrt)

    # ---- phase 0: LayerNorm stats from xT (rows, no transpose) ----
    rb_ctx = contextlib.ExitStack()
    rbp = rb_ctx.enter_context(tc.tile_pool(name="rb_p", bufs=1))
    R_b = rbp.tile([P, T], F32, tag="R_b")
    RMU_b = rbp.tile([P, T], F32, tag="RMU_b")
    BV_b = rbp.tile([P, D], F32, tag="BV_b")
    nc.gpsimd.partition_broadcast(out_ap=BV_b, in_ap=bv_row)
    with tc.tile_pool(name="sq_p", bufs=2) as sqp, \
         tc.tile_pool(name="strow", bufs=1) as strp, \
         tc.tile_pool(name="stps", bufs=1, space="PSUM") as stps:
        psS = [stps.tile([1, TF], F32, name=f"psS{i}", tag=f"psS{i}")
               for i in range(NTF)]
        psQ = [stps.tile([1, TF], F32, name=f"psQ{i}", tag=f"psQ{i}")
               for i in range(NTF)]
        for dc in range(ND):
            sq = sqp.tile([P, T], BF16, tag="sq")
            nc.scalar.activation(out=sq, in_=xn_sb[dc], func=Act.Square)
            for i in range(NTF):
                ts = slice(i * TF, (i + 1) * TF)
                nc.tensor.matmul(psS[i], lhsT=ones1, rhs=xn_sb[dc][:, ts],
                                 start=(dc == 0), stop=(dc == ND - 1))
                nc.tensor.matmul(psQ[i], lhsT=ones1, rhs=sq[:, ts],
                                 start=(dc == 0), stop=(dc == ND - 1))
        mu = strp.tile([1, T], F32, tag="mu")
        var = strp.tile([1, T], F32, tag="var")
        r_row = strp.tile([1, T], F32, tag="r_row")
        rmu_row = strp.tile([1, T], F32, tag="rmu_row")
        for i in range(NTF):
            ts = slice(i * TF, (i + 1) * TF)
            nc.vector.tensor_scalar_mul(out=mu[:, ts], in0=psS[i],
                                        scalar1=1.0 / D)
            # var = msq/D - mu^2  (two steps through var as scratch)
            nc.vector.tensor_mul(out=var[:, ts], in0=mu[:, ts], in1=mu[:, ts])
            nc.vector.scalar_tensor_tensor(
                out=var[:, ts], in0=psQ[i], scalar=1.0 / D, in1=var[:, ts],
                op0=Alu.mult, op1=Alu.subtract)
        # r = 1/sqrt(var+eps); rmu = -r*mu
        nc.scalar.activation(out=r_row, in_=var, func=Act.Sqrt, bias=eps_t)
        nc.vector.reciprocal(out=r_row, in_=r_row)
        nc.vector.scalar_tensor_tensor(
            out=rmu_row, in0=r_row, scalar=-1.0, in1=mu,
            op0=Alu.mult, op1=Alu.mult)
        nc.gpsimd.partition_broadcast(out_ap=R_b, in_ap=r_row)
        nc.gpsimd.partition_broadcast(out_ap=RMU_b, in_ap=rmu_row)

    # ---- phase 1: normalize x in place: xn = x*r + rmu ----
    for dc in range(ND):
        nc.vector.tensor_mul(out=xn_sb[dc], in0=xn_sb[dc], in1=R_b)
        nc.vector.tensor_add(out=xn_sb[dc], in0=xn_sb[dc], in1=RMU_b)

    # ---- phase 2: V projection, (t, j) layout, ones column per head ----
    v_pad_sb = []
    for tt in range(NT):
        v = singles.tile([P, H, hd + 1], BF16, name=f"vpad{tt}", tag=f"vpad{tt}")
        nc.vector.memset(v, 1.0)
        v_pad_sb.append(v)
    with tc.tile_pool(name="p2psum", bufs=3, space="PSUM") as p2ps:
        hpf = VF // hd  # heads covered per j chunk
        for tt in range(NT):
            for jf in range(NVF):
                js = slice(jf * VF, (jf + 1) * VF)
                ps = p2ps.tile([P, VF], F32, tag="ps")
                for dc in range(ND):
                    nc.tensor.matmul(
                        ps, lhsT=xn_sb[dc][:, tt * P:(tt + 1) * P],
                        rhs=wv_sb[dc][:, js],
                        start=(dc == 0), stop=(dc == ND - 1))
                nc.vector.tensor_add(
                    out=v_pad_sb[tt][:, jf * hpf:(jf + 1) * hpf, 0:hd],
                    in0=ps.rearrange("p (a b) -> p a b", b=hd),
                    in1=BV_b[:, js].rearrange("p (a b) -> p a b", b=hd))
    rb_ctx.close()
    wv_ctx.close()

    # ---- phases 3+4 software-pipelined per head-pair ----
    attn_sb = [singles.tile([P, T], BF16, name=f"attn{c}", tag=f"attn{c}")
               for c in range(NPAIR)]

    p1ps_ctx = contextlib.ExitStack()
    p1ps = p1ps_ctx.enter_context(tc.tile_pool(name="p1psum", bufs=2,
                                               space="PSUM"))
    scps = p1ps_ctx.enter_context(tc.tile_pool(name="scps", bufs=2,
                                               space="PSUM"))
    avps = p1ps_ctx.enter_context(tc.tile_pool(name="avps", bufs=2,
                                               space="PSUM"))
    qkp = p1ps_ctx.enter_context(tc.tile_pool(name="qkp", bufs=2))
    attp = p1ps_ctx.enter_context(tc.tile_pool(name="attp", bufs=2))
    lp = p1ps_ctx.enter_context(tc.tile_pool(name="lp", bufs=3))

    def phase_qk(c):
        """Project Q_c and K_c into (j, t) bf16 tiles; returns (q, k)."""
        tiles = {}
        for which, jc in (("q", c), ("k", NQ + c)):
            qk = qkp.tile([P, T], BF16, name=f"{which}T{c}", tag=f"{which}T")
            for tf in range(NTF):
                ts = slice(tf * TF, (tf + 1) * TF)
                ps = p1ps.tile([P, TF], F32, tag="ps")
                for dc in range(ND):
                    nc.tensor.matmul(
                        ps, lhsT=wqk_sb[dc][:, jc * P:(jc + 1) * P],
                        rhs=xn_sb[dc][:, ts],
                        start=(dc == 0), stop=(dc == ND - 1))
                nc.vector.tensor_scalar_add(out=qk[:, ts], in0=ps,
                                            scalar1=c2qk_sb[:, jc:jc + 1])
            tiles[which] = qk
        return tiles["q"], tiles["k"]

    def phase_scores(c, qtile, ktile):
        """scores -> exp (kv-mask bias) -> causal stripe; returns atts."""
        atts = {}
        for sc in range(NT):
            w = T - sc * P  # valid columns for this key block
            for h01 in (0, 1):
                hp = slice(h01 * hd, (h01 + 1) * hd)
                att = attp.tile([P, w], BF16, name=f"att{h01}_{sc}",
                                tag=f"att{h01}_{sc}")
                atts[(h01, sc)] = att
                for tf in range(NTF):
                    lo = max(sc * P, tf * TF)  # global first valid col
                    hi = (tf + 1) * TF
                    if lo >= hi:
                        continue
                    ps = scps.tile([P, TF], F32, name=f"scp{h01}",
                                   tag=f"scp{h01}")
                    nc.tensor.matmul(
                        ps[:, lo - tf * TF:TF],
                        lhsT=ktile[hp, sc * P:(sc + 1) * P],
                        rhs=qtile[hp, lo:hi],
                        start=True, stop=True,
                        tile_position=(h01 * hd, 0))
                    nc.scalar.activation(
                        out=att[:, lo - sc * P:hi - sc * P],
                        in_=ps[:, lo - tf * TF:TF], func=Act.Exp,
                        bias=kvm_sb[:, sc:sc + 1], scale=scale)
            for h01 in (0, 1):
                # diagonal stripe: zero strictly-upper within the block
                nc.gpsimd.affine_select(
                    out=atts[(h01, sc)][:, 0:P],
                    in_=atts[(h01, sc)][:, 0:P],
                    pattern=[[1, P]], compare_op=Alu.is_ge, fill=0.0,
                    base=0, channel_multiplier=-1)
        return atts

    def phase_av(c, atts):
        """att @ V (+ denominator), normalize into attn_sb[c]."""
        for tf in range(NTF):
            n_sc = min(NT, (tf + 1) * TF // P)
            for h01 in (0, 1):
                h = 2 * c + h01
                pso = avps.tile([hd + 1, TF], F32, tag="pso")
                for sc in range(n_sc):
                    lo = max(sc * P, tf * TF)
                    hi = (tf + 1) * TF
                    nc.tensor.matmul(
                        pso[:, lo - tf * TF:TF],
                        lhsT=v_pad_sb[sc][:, h, 0:hd + 1],
                        rhs=atts[(h01, sc)][:, lo - sc * P:hi - sc * P],
                        start=(sc == 0), stop=(sc == n_sc - 1))
                lrow = lp.tile([1, TF], F32, tag="lrow")
                nc.vector.tensor_copy(out=lrow, in_=pso[hd:hd + 1, :])
                nc.vector.reciprocal(out=lrow, in_=lrow)
                lb = lp.tile([hd, TF], F32, tag="lb")
                nc.gpsimd.partition_broadcast(out_ap=lb, in_ap=lrow)
                nc.vector.tensor_mul(
                    out=attn_sb[c][h01 * hd:(h01 + 1) * hd,
                                   tf * TF:(tf + 1) * TF],
                    in0=pso[0:hd, :], in1=lb)

    q0, k0 = phase_qk(0)
    prev = (0, phase_scores(0, q0, k0))
    for c in range(1, NPAIR):
        q, k = phase_qk(c)
        atts = phase_scores(c, q, k)
        phase_av(*prev)
        prev = (c, atts)
    phase_av(*prev)
    p1ps_ctx.close()
    wqk_ctx.close()

    # ---- phase 5: output projection, (e, t) layout ----
    with tc.tile_pool(name="p4psum", bufs=3, space="PSUM") as p4ps, \
         tc.tile_pool(name="p4tmp", bufs=3) as p4tmp:
        for ec in range(ND):
            for tf in range(NTF):
                ts = slice(tf * TF, (tf + 1) * TF)
                ps = p4ps.tile([P, TF], F32, tag="ps")
                for vc in range(ND):
                    nc.tensor.matmul(
                        ps, lhsT=wout_sb[vc][:, ec * P:(ec + 1) * P],
                        rhs=attn_sb[vc][:, ts],
                        start=(vc == 0), stop=(vc == ND - 1))
                ot = p4tmp.tile([P, TF], BF16, tag="ot")
                nc.scalar.activation(out=ot, in_=ps, func=Act.Identity,
                                     bias=bout_sb[:, ec:ec + 1])
                nc.sync.dma_start(out=out[ec * P:(ec + 1) * P, ts], in_=ot)
    ctx.close()


def host_inputs(xb, x_len, gamma, beta, w_qkv, b_qkv, w_out, b_out,
                T=1024, D=1024, H=16):
    """Build the per-core input map (numpy) for the bass program."""
    bf16 = ml_dtypes.bfloat16
    ND = D // P
    NT = T // P
    NJQK = 2 * D // P

    Wp = (gamma[:, None] * w_qkv).astype(np.float32)
    c2 = (beta @ w_qkv + b_qkv).astype(np.float32)

    xT_bf = np.ascontiguousarray(xb.T).astype(bf16)
    wqk_bf = np.ascontiguousarray(Wp[:, :2 * D]).astype(bf16)
    wv_bf = np.ascontiguousarray(Wp[:, 2 * D:]).astype(bf16)
    wout_bf = np.ascontiguousarray(w_out).astype(bf16)

    c2qk = np.ascontiguousarray(c2[:2 * D].reshape(NJQK, P).T).astype(np.float32)
    bv = np.ascontiguousarray(c2[2 * D:].reshape(1, D)).astype(np.float32)

    kv = np.where(np.arange(T) < int(x_len), 0.0, NEG).astype(np.float32)
    kvm = np.ascontiguousarray(kv.reshape(NT, P).T).astype(np.float32)

    bo = np.ascontiguousarray(b_out.reshape(ND, P).T).astype(np.float32)

    return {
        "xT": xT_bf,
        "wqk": wqk_bf, "wv": wv_bf, "wout": wout_bf,
        "c2qk": c2qk, "bv": bv,
        "kvm": kvm, "bout": bo,
    }


_COMPILED = {}


def _get_program():
    key = (T_FULL, D_FULL, H_FULL)
    if key not in _COMPILED:
        import concourse.tile as tile
        from concourse import bacc
        nc = bacc.Bacc("TRN2", target_bir_lowering=False, debug=False,
                       num_devices=B)
        with tile.TileContext(nc) as tc:
            build_attention(nc, tc, T=T_FULL, D=D_FULL, H=H_FULL, EPS=EPS)
        nc.compile()
        _COMPILED[key] = nc
    return _COMPILED[key]


def _run(inputs, trace=False):
    from concourse.bass_utils import run_bass_kernel_spmd

    x = np.asarray(inputs["x"], np.float32)
    x_lens = np.asarray(inputs["x_lens"])
    gamma = np.asarray(inputs["ln_gamma"], np.float32)
    beta = np.asarray(inputs["ln_beta"], np.float32)
    w_qkv = np.asarray(inputs["w_qkv"], np.float32)
    b_qkv = np.asarray(inputs["b_qkv"], np.float32)
    w_out = np.asarray(inputs["w_out"], np.float32)
    b_out = np.asarray(inputs["b_out"], np.float32)

    nc = _get_program()
    in_maps = [
        host_inputs(x[b], int(x_lens[b]), gamma, beta, w_qkv, b_qkv,
                    w_out, b_out, T=T_FULL, D=D_FULL, H=H_FULL)
        for b in range(B)
    ]
    res = run_bass_kernel_spmd(nc, in_maps, list(range(B)), trace=trace)
    out = np.stack([np.asarray(res.results[b]["out"]).astype(np.float32).T
                    for b in range(B)])
    return out, res


def kernel(**inputs):
    out, _ = _run(inputs, trace=False)
    return out


def kernel_traced(**inputs):
    """Like kernel() but also returns the SPMD run results (exec_time_ns...)."""
    import types
    try:
        from trn_agent_boot.trn_boot import _ntff_profile_via_ctypes
        hook = _ntff_profile_via_ctypes('/opt/axon/libaxon_pjrt.so')
        m = types.ModuleType('antenv.axon_hooks')
        m.get_axon_ntff_profile_hook = lambda: hook
        sys.modules.setdefault('antenv.axon_hooks', m)
    except Exception:
        pass
    out, res = _run(inputs, trace=True)
    return out, res


# revision 12
# speedup vs baseline: 1.2360x; 1.2360x over previous
"""Causal self-attention (LayerNorm + fused QKV + causal/len-masked softmax
attention + out-proj) on 8 Trainium2 NeuronCores, data-parallel over batch.

Contract: kernel(**inputs) takes the full unsharded inputs (B=8, T=1024,
D=1024, H=16) and returns the full (B, T, D) float32 output. Each core
processes one batch element; there are no cross-core collectives.

Device program per core (see build_attention):
  - LayerNorm stats from x^T via scalar-Square + ones-column matmuls into
    PSUM rows [1, T]; r = 1/sqrt(var+eps) and rmu = -r*mu stay in row form
    and are partition-broadcast on GpSimd (no DRAM scratch round-trips).
  - x is normalized in place (xn = x*r + rmu), so Q/K/V projections are
    pure matmuls; biases are fused into the PSUM evacuations.
  - Q^T/K^T in (j, t) layout, V in (t, j) layout with a ones column per
    head so att@V also produces the softmax denominator.
  - scores^T per head with K=64 row-packed matmul pairs (tile_position);
    exp on the scalar engine with the kv-length mask as per-partition
    bias; causal masking via gpsimd affine_select on diagonal blocks.
  - Softmax denominator: reciprocal of the PSUM l-row on vector, then
    GpSimd partition_broadcast, then one fused multiply during evacuation.
  - Out-proj emits out^T in bf16; the host casts/transposes back.
"""

import math
import sys

for _p in ('/opt/trn_rl_repo', '/opt/trn_rl_repo/pypackages', '/root/.axon_site'):
    if _p not in sys.path:
        sys.path.insert(0, _p)

import numpy as np
import ml_dtypes

import concourse.bass as bass
import concourse.mybir as mybir

dt = mybir.dt
F32 = dt.float32
BF16 = dt.bfloat16
Alu = mybir.AluOpType
Act = mybir.ActivationFunctionType

P = 128
B, T_FULL, D_FULL, H_FULL = 8, 1024, 1024, 16
NEG = -1e9
EPS = 1e-5


def build_attention(nc, tc, T=1024, D=1024, H=16, EPS=1e-5):
    hd = D // H
    assert hd == 64, "row-packed scores assume head_dim == 64"
    ND = D // P              # d-chunks (contraction for projections)
    NT = T // P              # t-chunks of 128 (s-chunks too)
    TF = 512                 # free-dim t chunk (psum bank limit for f32)
    NTF = T // TF
    NQ = D // P              # number of Q chunks (K chunks follow)
    VF = 512                 # j free-chunk width for V
    NVF = D // VF
    NPAIR = H // 2
    scale = 1.0 / math.sqrt(hd)

    # ---- DRAM parameters ----
    xT = nc.declare_dram_parameter("xT", [D, T], BF16, isOutput=False)
    wqk = nc.declare_dram_parameter("wqk", [D, 2 * D], BF16, isOutput=False)
    wv = nc.declare_dram_parameter("wv", [D, D], BF16, isOutput=False)
    wout = nc.declare_dram_parameter("wout", [D, D], BF16, isOutput=False)
    c2qk = nc.declare_dram_parameter("c2qk", [P, 2 * NQ], F32, isOutput=False)
    bv = nc.declare_dram_parameter("bv", [1, D], F32, isOutput=False)
    kvm = nc.declare_dram_parameter("kvm", [P, NT], F32, isOutput=False)
    bout = nc.declare_dram_parameter("bout", [P, ND], F32, isOutput=False)
    out = nc.declare_dram_parameter("out", [D, T], BF16, isOutput=True)

    import contextlib
    ctx = contextlib.ExitStack()
    singles = ctx.enter_context(tc.tile_pool(name="singles", bufs=1))

    # ---- standing SBUF tiles; DMA order: xT, wv, consts, wqk, wout ----
    # Pools are a strict stack: create in reverse-lifetime order
    # (wout > wqk > wv/rb), emit DMAs in arrival-priority order.
    wop = ctx.enter_context(tc.tile_pool(name="wout_p", bufs=1))
    wqk_ctx = contextlib.ExitStack()
    wqkp = wqk_ctx.enter_context(tc.tile_pool(name="wqk_p", bufs=1))
    wv_ctx = contextlib.ExitStack()
    wvp = wv_ctx.enter_context(tc.tile_pool(name="wv_p", bufs=1))

    xn_sb = []
    for dc in range(ND):
        t = singles.tile([P, T], BF16, name=f"xn{dc}", tag=f"xn{dc}")
        nc.sync.dma_start(out=t, in_=xT[dc * P:(dc + 1) * P, :])
        xn_sb.append(t)
    wv_sb = []
    for dc in range(ND):
        w = wvp.tile([P, D], BF16, name=f"wv{dc}", tag=f"wv{dc}")
        nc.sync.dma_start(out=w, in_=wv[dc * P:(dc + 1) * P, :])
        wv_sb.append(w)
    c2qk_sb = singles.tile([P, 2 * NQ], F32, tag="c2qk")
    nc.sync.dma_start(out=c2qk_sb, in_=c2qk[:, :])
    bv_row = singles.tile([1, D], F32, tag="bv_row")
    nc.sync.dma_start(out=bv_row, in_=bv[:, :])
    kvm_sb = singles.tile([P, NT], F32, tag="kvm")
    nc.sync.dma_start(out=kvm_sb, in_=kvm[:, :])
    bout_sb = singles.tile([P, ND], F32, tag="bout")
    nc.sync.dma_start(out=bout_sb, in_=bout[:, :])
    wqk_sb = []
    for dc in range(ND):
        w = wqkp.tile([P, 2 * D], BF16, name=f"wqk{dc}", tag=f"wqk{dc}")
        nc.sync.dma_start(out=w, in_=wqk[dc * P:(dc + 1) * P, :])
        wqk_sb.append(w)
    wout_sb = []
    for vc in range(ND):
        w = wop.tile([P, D], BF16, name=f"wout{vc}", tag=f"wout{vc}")
        nc.sync.dma_start(out=w, in_=wout[vc * P:(vc + 1) * P, :])
        wout_sb.append(w)

    ones1 = singles.tile([P, 1], BF16, tag="ones1")
    nc.vector.memset(ones1, 1.0)
    eps_t = singles.tile([1, 1], F32, tag="eps")
    nc.vector.memset(eps_t, EPS)
    # Load the sqrt act table before the squares so only one later switch
    # (to the exp table) happens.
    sqd = singles.tile([1, 1], F32, tag="sqd")
    nc.scalar.activation(out=sqd, in_=eps_t, func=Act.Sqrt)

    # ---- phase 0: LayerNorm stats from xT (rows, no transpose) ----
    rb_ctx = contextlib.ExitStack()
    rbp = rb_ctx.enter_context(tc.tile_pool(name="rb_p", bufs=1))
    R_b = rbp.tile([P, T], F32, tag="R_b")
    RMU_b = rbp.tile([P, T], F32, tag="RMU_b")
    BV_b = rbp.tile([P, D], F32, tag="BV_b")
    nc.gpsimd.partition_broadcast(out_ap=BV_b, in_ap=bv_row)
    with tc.tile_pool(name="sq_p", bufs=2) as sqp, \
         tc.tile_pool(name="strow", bufs=1) as strp, \
         tc.tile_pool(name="stps", bufs=1, space="PSUM") as stps:
        psS = [stps.tile([1, TF], F32, name=f"psS{i}", tag=f"psS{i}")
               for i in range(NTF)]
        psQ = [stps.tile([1, TF], F32, name=f"psQ{i}", tag=f"psQ{i}")
               for i in range(NTF)]
        for dc in range(ND):
            sq = sqp.tile([P, T], BF16, tag="sq")
            nc.scalar.activation(out=sq, in_=xn_sb[dc], func=Act.Square)
            for i in range(NTF):
                ts = slice(i * TF, (i + 1) * TF)
                nc.tensor.matmul(psS[i], lhsT=ones1, rhs=xn_sb[dc][:, ts],
                                 start=(dc == 0), stop=(dc == ND - 1))
                nc.tensor.matmul(psQ[i], lhsT=ones1, rhs=sq[:, ts],
                                 start=(dc == 0), stop=(dc == ND - 1))
        mu = strp.tile([1, T], F32, tag="mu")
        var = strp.tile([1, T], F32, tag="var")
        r_row = strp.tile([1, T], F32, tag="r_row")
        rmu_row = strp.tile([1, T], F32, tag="rmu_row")
        for i in range(NTF):
            ts = slice(i * TF, (i + 1) * TF)
            nc.vector.tensor_scalar_mul(out=mu[:, ts], in0=psS[i],
                                        scalar1=1.0 / D)
            # var = msq/D - mu^2  (two steps through var as scratch)
            nc.vector.tensor_mul(out=var[:, ts], in0=mu[:, ts], in1=mu[:, ts])
            nc.vector.scalar_tensor_tensor(
                out=var[:, ts], in0=psQ[i], scalar=1.0 / D, in1=var[:, ts],
                op0=Alu.mult, op1=Alu.subtract)
        # r = 1/sqrt(var+eps); rmu = -r*mu
        nc.scalar.activation(out=r_row, in_=var, func=Act.Sqrt, bias=eps_t)
        nc.vector.reciprocal(out=r_row, in_=r_row)
        nc.vector.scalar_tensor_tensor(
            out=rmu_row, in0=r_row, scalar=-1.0, in1=mu,
            op0=Alu.mult, op1=Alu.mult)
        nc.gpsimd.partition_broadcast(out_ap=R_b, in_ap=r_row)
        nc.gpsimd.partition_broadcast(out_ap=RMU_b, in_ap=rmu_row)

    # ---- phase 1: normalize x in place: xn = x*r + rmu ----
    # mult on vector, add on gpsimd so the two passes pipeline.
    for dc in range(ND):
        nc.vector.tensor_mul(out=xn_sb[dc], in0=xn_sb[dc], in1=R_b)
        nc.gpsimd.tensor_add(out=xn_sb[dc], in0=xn_sb[dc], in1=RMU_b)

    # ---- phase 2: V projection, (t, j) layout, ones column per head ----
    v_pad_sb = []
    for tt in range(NT):
        v = singles.tile([P, H, hd + 1], BF16, name=f"vpad{tt}", tag=f"vpad{tt}")
        nc.vector.memset(v, 1.0)
        v_pad_sb.append(v)
    with tc.tile_pool(name="p2psum", bufs=3, space="PSUM") as p2ps:
        hpf = VF // hd  # heads covered per j chunk
        for tt in range(NT):
            # jf-inner so consecutive matmuls share the stationary operand
            pss = [p2ps.tile([P, VF], F32, name=f"ps{jf}", tag="ps")
                   for jf in range(NVF)]
            for dc in range(ND):
                for jf in range(NVF):
                    nc.tensor.matmul(
                        pss[jf], lhsT=xn_sb[dc][:, tt * P:(tt + 1) * P],
                        rhs=wv_sb[dc][:, jf * VF:(jf + 1) * VF],
                        start=(dc == 0), stop=(dc == ND - 1))
            for jf in range(NVF):
                js = slice(jf * VF, (jf + 1) * VF)
                nc.vector.tensor_add(
                    out=v_pad_sb[tt][:, jf * hpf:(jf + 1) * hpf, 0:hd],
                    in0=pss[jf].rearrange("p (a b) -> p a b", b=hd),
                    in1=BV_b[:, js].rearrange("p (a b) -> p a b", b=hd))
    rb_ctx.close()
    wv_ctx.close()

    # ---- phases 3+4 software-pipelined per head-pair ----
    attn_sb = [singles.tile([P, T], BF16, name=f"attn{c}", tag=f"attn{c}")
               for c in range(NPAIR)]

    p1ps_ctx = contextlib.ExitStack()
    p1ps = p1ps_ctx.enter_context(tc.tile_pool(name="p1psum", bufs=2,
                                               space="PSUM"))
    scps = p1ps_ctx.enter_context(tc.tile_pool(name="scps", bufs=2,
                                               space="PSUM"))
    avps = p1ps_ctx.enter_context(tc.tile_pool(name="avps", bufs=2,
                                               space="PSUM"))
    qkp = p1ps_ctx.enter_context(tc.tile_pool(name="qkp", bufs=2))
    attp = p1ps_ctx.enter_context(tc.tile_pool(name="attp", bufs=2))
    lp = p1ps_ctx.enter_context(tc.tile_pool(name="lp", bufs=3))

    def phase_qk(c):
        """Project Q_c and K_c into (j, t) bf16 tiles; returns (q, k)."""
        tiles = {}
        for which, jc in (("q", c), ("k", NQ + c)):
            qk = qkp.tile([P, T], BF16, name=f"{which}T{c}", tag=f"{which}T")
            # tf-inner so consecutive matmuls share the stationary operand
            pss = [p1ps.tile([P, TF], F32, name=f"ps{tf}", tag="ps")
                   for tf in range(NTF)]
            for dc in range(ND):
                for tf in range(NTF):
                    nc.tensor.matmul(
                        pss[tf], lhsT=wqk_sb[dc][:, jc * P:(jc + 1) * P],
                        rhs=xn_sb[dc][:, tf * TF:(tf + 1) * TF],
                        start=(dc == 0), stop=(dc == ND - 1))
            for tf in range(NTF):
                ts = slice(tf * TF, (tf + 1) * TF)
                nc.vector.tensor_scalar_add(out=qk[:, ts], in0=pss[tf],
                                            scalar1=c2qk_sb[:, jc:jc + 1])
            tiles[which] = qk
        return tiles["q"], tiles["k"]

    def phase_scores(c, qtile, ktile):
        """scores -> exp (kv-mask bias) -> causal stripe; returns atts."""
        atts = {}
        for sc in range(NT):
            w = T - sc * P  # valid columns for this key block
            for h01 in (0, 1):
                hp = slice(h01 * hd, (h01 + 1) * hd)
                att = attp.tile([P, w], BF16, name=f"att{h01}_{sc}",
                                tag=f"att{h01}_{sc}")
                atts[(h01, sc)] = att
                # both tf windows share the stationary K block: MMs
                # back-to-back, exps after.
                work = []
                for tf in range(NTF):
                    lo = max(sc * P, tf * TF)  # global first valid col
                    hi = (tf + 1) * TF
                    if lo >= hi:
                        continue
                    ps = scps.tile([P, TF], F32, name=f"scp{h01}_{tf}",
                                   tag=f"scp{h01}")
                    nc.tensor.matmul(
                        ps[:, lo - tf * TF:TF],
                        lhsT=ktile[hp, sc * P:(sc + 1) * P],
                        rhs=qtile[hp, lo:hi],
                        start=True, stop=True,
                        tile_position=(h01 * hd, 0))
                    work.append((ps, lo, hi, tf))
                for ps, lo, hi, tf in work:
                    nc.scalar.activation(
                        out=att[:, lo - sc * P:hi - sc * P],
                        in_=ps[:, lo - tf * TF:TF], func=Act.Exp,
                        bias=kvm_sb[:, sc:sc + 1], scale=scale)
            for h01 in (0, 1):
                # diagonal stripe: zero strictly-upper within the block
                nc.gpsimd.affine_select(
                    out=atts[(h01, sc)][:, 0:P],
                    in_=atts[(h01, sc)][:, 0:P],
                    pattern=[[1, P]], compare_op=Alu.is_ge, fill=0.0,
                    base=0, channel_multiplier=-1)
        return atts

    def phase_av(c, atts):
        """att @ V (+ denominator), normalize into attn_sb[c].

        tf-inner so the two psum accumulations share each stationary V
        block; pso is evacuated to SBUF immediately (frees the bank), the
        slow recip/broadcast chain then runs out of SBUF.
        """
        for h01 in (0, 1):
            h = 2 * c + h01
            psos = [avps.tile([hd + 1, TF], F32, name=f"pso{tf}", tag="pso")
                    for tf in range(NTF)]
            for sc in range(NT):
                for tf in range(NTF):
                    lo = max(sc * P, tf * TF)
                    hi = (tf + 1) * TF
                    if lo >= hi:
                        continue
                    n_sc = min(NT, (tf + 1) * TF // P)
                    nc.tensor.matmul(
                        psos[tf][:, lo - tf * TF:TF],
                        lhsT=v_pad_sb[sc][:, h, 0:hd + 1],
                        rhs=atts[(h01, sc)][:, lo - sc * P:hi - sc * P],
                        start=(sc == 0), stop=(sc == n_sc - 1))
            for tf in range(NTF):
                po = lp.tile([hd + 1, TF], F32, name=f"po{tf}", tag="po")
                nc.vector.tensor_copy(out=po, in_=psos[tf])
                nc.vector.reciprocal_approx_fast(out=po[hd:hd + 1, :],
                                                 in_=po[hd:hd + 1, :])
                lb = lp.tile([hd, TF], F32, name=f"lb{tf}", tag="lb")
                nc.gpsimd.partition_broadcast(out_ap=lb, in_ap=po[hd:hd + 1, :])
                nc.vector.tensor_mul(
                    out=attn_sb[c][h01 * hd:(h01 + 1) * hd,
                                   tf * TF:(tf + 1) * TF],
                    in0=po[0:hd, :], in1=lb)

    q0, k0 = phase_qk(0)
    prev = (0, phase_scores(0, q0, k0))
    for c in range(1, NPAIR):
        q, k = phase_qk(c)
        atts = phase_scores(c, q, k)
        phase_av(*prev)
        prev = (c, atts)
    phase_av(*prev)
    p1ps_ctx.close()
    wqk_ctx.close()

    # ---- phase 5: output projection, (e, t) layout ----
    with tc.tile_pool(name="p4psum", bufs=4, space="PSUM") as p4ps, \
         tc.tile_pool(name="p4tmp", bufs=3) as p4tmp:
        for ec in range(ND):
            pss = [p4ps.tile([P, TF], F32, name=f"ps{tf}", tag="ps")
                   for tf in range(NTF)]
            for vc in range(ND):
                for tf in range(NTF):
                    nc.tensor.matmul(
                        pss[tf], lhsT=wout_sb[vc][:, ec * P:(ec + 1) * P],
                        rhs=attn_sb[vc][:, tf * TF:(tf + 1) * TF],
                        start=(vc == 0), stop=(vc == ND - 1))
            for tf in range(NTF):
                ts = slice(tf * TF, (tf + 1) * TF)
                ot = p4tmp.tile([P, TF], BF16, name=f"ot{tf}", tag="ot")
                nc.scalar.activation(out=ot, in_=pss[tf], func=Act.Identity,
                                     bias=bout_sb[:, ec:ec + 1])
                nc.sync.dma_start(out=out[ec * P:(ec + 1) * P, ts], in_=ot)
    ctx.close()


def host_inputs(xb, x_len, gamma, beta, w_qkv, b_qkv, w_out, b_out,
                T=1024, D=1024, H=16):
    """Build the per-core input map (numpy) for the bass program."""
    bf16 = ml_dtypes.bfloat16
    ND = D // P
    NT = T // P
    NJQK = 2 * D // P

    Wp = (gamma[:, None] * w_qkv).astype(np.float32)
    c2 = (beta @ w_qkv + b_qkv).astype(np.float32)

    xT_bf = np.ascontiguousarray(xb.T).astype(bf16)
    wqk_bf = np.ascontiguousarray(Wp[:, :2 * D]).astype(bf16)
    wv_bf = np.ascontiguousarray(Wp[:, 2 * D:]).astype(bf16)
    wout_bf = np.ascontiguousarray(w_out).astype(bf16)

    c2qk = np.ascontiguousarray(c2[:2 * D].reshape(NJQK, P).T).astype(np.float32)
    bv = np.ascontiguousarray(c2[2 * D:].reshape(1, D)).astype(np.float32)

    kv = np.where(np.arange(T) < int(x_len), 0.0, NEG).astype(np.float32)
    kvm = np.ascontiguousarray(kv.reshape(NT, P).T).astype(np.float32)

    bo = np.ascontiguousarray(b_out.reshape(ND, P).T).astype(np.float32)

    return {
        "xT": xT_bf,
        "wqk": wqk_bf, "wv": wv_bf, "wout": wout_bf,
        "c2qk": c2qk, "bv": bv,
        "kvm": kvm, "bout": bo,
    }


_COMPILED = {}


def _get_program():
    key = (T_FULL, D_FULL, H_FULL)
    if key not in _COMPILED:
        import concourse.tile as tile
        from concourse import bacc
        nc = bacc.Bacc("TRN2", target_bir_lowering=False, debug=False,
                       num_devices=B)
        with tile.TileContext(nc) as tc:
            build_attention(nc, tc, T=T_FULL, D=D_FULL, H=H_FULL, EPS=EPS)
        nc.compile()
        _COMPILED[key] = nc
    return _COMPILED[key]


def _run(inputs, trace=False):
    from concourse.bass_utils import run_bass_kernel_spmd

    x = np.asarray(inputs["x"], np.float32)
    x_lens = np.asarray(inputs["x_lens"])
    gamma = np.asarray(inputs["ln_gamma"], np.float32)
    beta = np.asarray(inputs["ln_beta"], np.float32)
    w_qkv = np.asarray(inputs["w_qkv"], np.float32)
    b_qkv = np.asarray(inputs["b_qkv"], np.float32)
    w_out = np.asarray(inputs["w_out"], np.float32)
    b_out = np.asarray(inputs["b_out"], np.float32)

    nc = _get_program()
    in_maps = [
        host_inputs(x[b], int(x_lens[b]), gamma, beta, w_qkv, b_qkv,
                    w_out, b_out, T=T_FULL, D=D_FULL, H=H_FULL)
        for b in range(B)
    ]
    res = run_bass_kernel_spmd(nc, in_maps, list(range(B)), trace=trace)
    out = np.stack([np.asarray(res.results[b]["out"]).astype(np.float32).T
                    for b in range(B)])
    return out, res


def kernel(**inputs):
    out, _ = _run(inputs, trace=False)
    return out


def kernel_traced(**inputs):
    """Like kernel() but also returns the SPMD run results (exec_time_ns...)."""
    import types
    try:
        from trn_agent_boot.trn_boot import _ntff_profile_via_ctypes
        hook = _ntff_profile_via_ctypes('/opt/axon/libaxon_pjrt.so')
        m = types.ModuleType('antenv.axon_hooks')
        m.get_axon_ntff_profile_hook = lambda: hook
        sys.modules.setdefault('antenv.axon_hooks', m)
    except Exception:
        pass
    out, res = _run(inputs, trace=True)
    return out, res


# revision 24
# speedup vs baseline: 1.3527x; 1.0944x over previous
"""Causal self-attention (LayerNorm + fused QKV + causal/len-masked softmax
attention + out-proj) on 8 Trainium2 NeuronCores, data-parallel over batch.

Contract: kernel(**inputs) takes the full unsharded inputs (B=8, T=1024,
D=1024, H=16) and returns the full (B, T, D) float32 output. Each core
processes one batch element; there are no cross-core collectives.

Device program per core (see build_attention):
  - LayerNorm stats from x^T via scalar-Square + ones-column matmuls into
    PSUM rows [1, T]; r = 1/sqrt(var+eps) and rmu = -r*mu stay in row form
    and are partition-broadcast on GpSimd (no DRAM scratch round-trips).
  - x is normalized in place (xn = x*r + rmu), so Q/K/V projections are
    pure matmuls; biases are fused into the PSUM evacuations.
  - Q^T/K^T in (j, t) layout, V in (t, j) layout with a ones column per
    head so att@V also produces the softmax denominator.
  - scores^T per head with K=64 row-packed matmul pairs (tile_position);
    exp on the scalar engine with the kv-length mask as per-partition
    bias; causal masking via gpsimd affine_select on diagonal blocks.
  - Softmax denominator: reciprocal of the PSUM l-row on vector, then
    GpSimd partition_broadcast, then one fused multiply during evacuation.
  - Out-proj emits out^T in bf16; the host casts/transposes back.
"""

import math
import sys

for _p in ('/opt/trn_rl_repo', '/opt/trn_rl_repo/pypackages', '/root/.axon_site'):
    if _p not in sys.path:
        sys.path.insert(0, _p)

import numpy as np
import ml_dtypes

import concourse.bass as bass
import concourse.mybir as mybir

dt = mybir.dt
F32 = dt.float32
BF16 = dt.bfloat16
Alu = mybir.AluOpType
Act = mybir.ActivationFunctionType

P = 128
B, T_FULL, D_FULL, H_FULL = 8, 1024, 1024, 16
NEG = -1e9
EPS = 1e-5


def build_attention(nc, tc, T=1024, D=1024, H=16, EPS=1e-5):
    hd = D // H
    assert hd == 64, "row-packed scores assume head_dim == 64"
    ND = D // P              # d-chunks (contraction for projections)
    NT = T // P              # t-chunks of 128 (s-chunks too)
    TF = 512                 # free-dim t chunk (psum bank limit for f32)
    NTF = T // TF
    NQ = D // P              # number of Q chunks (K chunks follow)
    VF = 512                 # j free-chunk width for V
    NVF = D // VF
    NPAIR = H // 2
    scale = 1.0 / math.sqrt(hd)

    # ---- DRAM parameters ----
    xT = nc.declare_dram_parameter("xT", [D, T], BF16, isOutput=False)
    wqk = nc.declare_dram_parameter("wqk", [D, 2 * D], BF16, isOutput=False)
    wv = nc.declare_dram_parameter("wv", [D, D], BF16, isOutput=False)
    wout = nc.declare_dram_parameter("wout", [D, D], BF16, isOutput=False)
    c2qk = nc.declare_dram_parameter("c2qk", [P, 2 * NQ], F32, isOutput=False)
    c1v = nc.declare_dram_parameter("c1v", [1, D], F32, isOutput=False)
    bv = nc.declare_dram_parameter("bv", [1, D], F32, isOutput=False)
    kvm = nc.declare_dram_parameter("kvm", [P, NT], F32, isOutput=False)
    bout = nc.declare_dram_parameter("bout", [P, ND], F32, isOutput=False)
    out = nc.declare_dram_parameter("out", [D, T], BF16, isOutput=True)

    import contextlib
    ctx = contextlib.ExitStack()
    singles = ctx.enter_context(tc.tile_pool(name="singles", bufs=1))

    # ---- standing SBUF tiles; DMA order: xT, wv, consts, wqk, wout ----
    # Pools are a strict stack: create in reverse-lifetime order
    # (wout > wqk > wv/rb), emit DMAs in arrival-priority order.
    wop = ctx.enter_context(tc.tile_pool(name="wout_p", bufs=1))
    wqk_ctx = contextlib.ExitStack()
    wqkp = wqk_ctx.enter_context(tc.tile_pool(name="wqk_p", bufs=1))
    wv_ctx = contextlib.ExitStack()
    wvp = wv_ctx.enter_context(tc.tile_pool(name="wv_p", bufs=1))

    xn_sb = []
    for dc in range(ND):
        t = singles.tile([P, T], BF16, name=f"xn{dc}", tag=f"xn{dc}")
        nc.sync.dma_start(out=t, in_=xT[dc * P:(dc + 1) * P, :])
        xn_sb.append(t)
    wv_sb = []
    for dc in range(ND):
        w = wvp.tile([P, D], BF16, name=f"wv{dc}", tag=f"wv{dc}")
        nc.sync.dma_start(out=w, in_=wv[dc * P:(dc + 1) * P, :])
        wv_sb.append(w)
    c2qk_sb = singles.tile([P, 2 * NQ], F32, tag="c2qk")
    nc.sync.dma_start(out=c2qk_sb, in_=c2qk[:, :])
    c1v_row = singles.tile([1, D], F32, tag="c1v_row")
    nc.sync.dma_start(out=c1v_row, in_=c1v[:, :])
    bv_row = singles.tile([1, D], F32, tag="bv_row")
    nc.sync.dma_start(out=bv_row, in_=bv[:, :])
    kvm_sb = singles.tile([P, NT], F32, tag="kvm")
    nc.sync.dma_start(out=kvm_sb, in_=kvm[:, :])
    bout_sb = singles.tile([P, ND], F32, tag="bout")
    nc.sync.dma_start(out=bout_sb, in_=bout[:, :])
    wqk_sb = []
    for dc in range(ND):
        w = wqkp.tile([P, 2 * D], BF16, name=f"wqk{dc}", tag=f"wqk{dc}")
        nc.sync.dma_start(out=w, in_=wqk[dc * P:(dc + 1) * P, :])
        wqk_sb.append(w)
    wout_sb = []
    for vc in range(ND):
        w = wop.tile([P, D], BF16, name=f"wout{vc}", tag=f"wout{vc}")
        nc.sync.dma_start(out=w, in_=wout[vc * P:(vc + 1) * P, :])
        wout_sb.append(w)

    ones1 = singles.tile([P, 1], BF16, tag="ones1")
    nc.vector.memset(ones1, 1.0)
    eps_t = singles.tile([1, 1], F32, tag="eps")
    nc.vector.memset(eps_t, EPS)
    id1 = singles.tile([1, 1], F32, tag="id1")
    nc.vector.memset(id1, 1.0)
    # Load the sqrt act table before the squares so only one later switch
    # (to the exp table) happens.
    sqd = singles.tile([1, 1], F32, tag="sqd")
    nc.scalar.activation(out=sqd, in_=eps_t, func=Act.Sqrt)

    # ---- phase 0: LayerNorm stats from xT (rows, no DRAM round-trip) ----
    # r/rmu come out in row form [1, T] (for the xn pre-scale feeding Q/K)
    # and in column form [P, 1]-per-block via tiny PE transposes (for the
    # V-phase per-partition correction).
    rb_ctx = contextlib.ExitStack()
    rbp = rb_ctx.enter_context(tc.tile_pool(name="rb_p", bufs=1))
    R_b = rbp.tile([P, T], BF16, tag="R_b")
    RMU_b = rbp.tile([P, T], BF16, tag="RMU_b")
    C1V_b = rbp.tile([P, D], BF16, tag="C1V_b")
    C2V_b = rbp.tile([P, D], F32, tag="C2V_b")
    rcol = singles.tile([P, 2 * NT], F32, tag="rcol")
    with tc.tile_pool(name="sq_p", bufs=2) as sqp, \
         tc.tile_pool(name="strow", bufs=1) as strp, \
         tc.tile_pool(name="stps", bufs=1, space="PSUM") as stps:
        psS = [stps.tile([1, TF], F32, name=f"psS{i}", tag=f"psS{i}")
               for i in range(NTF)]
        psQ = [stps.tile([1, TF], F32, name=f"psQ{i}", tag=f"psQ{i}")
               for i in range(NTF)]
        for dc in range(ND):
            sq = sqp.tile([P, T], BF16, tag="sq")
            nc.scalar.activation(out=sq, in_=xn_sb[dc], func=Act.Square)
            for i in range(NTF):
                ts = slice(i * TF, (i + 1) * TF)
                nc.tensor.matmul(psS[i], lhsT=ones1, rhs=xn_sb[dc][:, ts],
                                 start=(dc == 0), stop=(dc == ND - 1))
                nc.tensor.matmul(psQ[i], lhsT=ones1, rhs=sq[:, ts],
                                 start=(dc == 0), stop=(dc == ND - 1))
        mu = strp.tile([1, T], F32, tag="mu")
        var = strp.tile([1, T], F32, tag="var")
        r_row = strp.tile([1, T], F32, tag="r_row")
        rmu_row = strp.tile([1, T], F32, tag="rmu_row")
        for i in range(NTF):
            ts = slice(i * TF, (i + 1) * TF)
            nc.vector.tensor_scalar_mul(out=mu[:, ts], in0=psS[i],
                                        scalar1=1.0 / D)
            # var = msq/D - mu^2  (two steps through var as scratch)
            nc.vector.tensor_mul(out=var[:, ts], in0=mu[:, ts], in1=mu[:, ts])
            nc.vector.scalar_tensor_tensor(
                out=var[:, ts], in0=psQ[i], scalar=1.0 / D, in1=var[:, ts],
                op0=Alu.mult, op1=Alu.subtract)
        # r = 1/sqrt(var+eps); rmu = -r*mu
        nc.scalar.activation(out=r_row, in_=var, func=Act.Sqrt, bias=eps_t)
        nc.vector.reciprocal_approx_fast(out=r_row, in_=r_row)
        nc.vector.scalar_tensor_tensor(
            out=rmu_row, in0=r_row, scalar=-1.0, in1=mu,
            op0=Alu.mult, op1=Alu.mult)
        # column form via tiny PE transposes: rcol[:, tt] = r block tt,
        # rcol[:, NT+tt] = rmu block tt
        with tc.tile_pool(name="trps", bufs=1, space="PSUM") as trps:
            pst = trps.tile([P, 2 * NT], F32, tag="pst")
            for tt in range(NT):
                nc.tensor.transpose(pst[:, tt:tt + 1],
                                    in_=r_row[:, tt * P:(tt + 1) * P],
                                    identity=id1)
                nc.tensor.transpose(pst[:, NT + tt:NT + tt + 1],
                                    in_=rmu_row[:, tt * P:(tt + 1) * P],
                                    identity=id1)
            nc.vector.tensor_copy(out=rcol, in_=pst)
        # row form (bf16) broadcast for the xn pre-scale
        r_bf = strp.tile([1, T], BF16, tag="r_bf")
        nc.vector.tensor_copy(out=r_bf, in_=r_row)
        rmu_bf = strp.tile([1, T], BF16, tag="rmu_bf")
        nc.vector.tensor_copy(out=rmu_bf, in_=rmu_row)
        nc.gpsimd.partition_broadcast(out_ap=R_b, in_ap=r_bf)
        nc.gpsimd.partition_broadcast(out_ap=RMU_b, in_ap=rmu_bf)
        c1v_bf = strp.tile([1, D], BF16, tag="c1v_bf")
        nc.vector.tensor_copy(out=c1v_bf, in_=c1v_row)
        nc.gpsimd.partition_broadcast(out_ap=C1V_b, in_ap=c1v_bf)
        nc.gpsimd.partition_broadcast(out_ap=C2V_b, in_ap=bv_row)

    # ---- phase 1: xn = x*r + rmu into separate tiles (xT stays raw for
    # the V projection, which must not wait on this chain) ----
    xr_sb = []
    for dc in range(ND):
        xr = singles.tile([P, T], BF16, name=f"xr{dc}", tag=f"xr{dc}")
        nc.vector.tensor_mul(out=xr, in0=xn_sb[dc], in1=R_b)
        nc.vector.tensor_add(out=xr, in0=xr, in1=RMU_b)
        xr_sb.append(xr)

    # ---- phase 2: V projection from RAW xT, (t, j) layout, ones column;
    # LayerNorm correction applied per-partition on evacuation ----
    v_pad_sb = []
    for tt in range(NT):
        v = singles.tile([P, H, hd + 1], BF16, name=f"vpad{tt}", tag=f"vpad{tt}")
        nc.vector.memset(v, 1.0)
        v_pad_sb.append(v)
    with tc.tile_pool(name="p2psum", bufs=3, space="PSUM") as p2ps, \
         tc.tile_pool(name="s2vp", bufs=3) as s2vp:
        hpf = VF // hd  # heads covered per j chunk
        for tt in range(NT):
            # jf-inner so consecutive matmuls share the stationary operand
            pss = [p2ps.tile([P, VF], F32, name=f"ps{jf}", tag="ps")
                   for jf in range(NVF)]
            for dc in range(ND):
                for jf in range(NVF):
                    nc.tensor.matmul(
                        pss[jf], lhsT=xn_sb[dc][:, tt * P:(tt + 1) * P],
                        rhs=wv_sb[dc][:, jf * VF:(jf + 1) * VF],
                        start=(dc == 0), stop=(dc == ND - 1))
            for jf in range(NVF):
                js = slice(jf * VF, (jf + 1) * VF)
                # s2v = rmu(t)*c1v(j) + c2v(j); v = r(t)*psv + s2v
                s2v = s2vp.tile([P, VF], F32, name=f"s2v{jf}", tag="s2v")
                nc.vector.scalar_tensor_tensor(
                    out=s2v, in0=C1V_b[:, js],
                    scalar=rcol[:, NT + tt:NT + tt + 1],
                    in1=C2V_b[:, js], op0=Alu.mult, op1=Alu.add)
                nc.vector.scalar_tensor_tensor(
                    out=v_pad_sb[tt][:, jf * hpf:(jf + 1) * hpf, 0:hd],
                    in0=pss[jf].rearrange("p (a b) -> p a b", b=hd),
                    scalar=rcol[:, tt:tt + 1],
                    in1=s2v.rearrange("p (a b) -> p a b", b=hd),
                    op0=Alu.mult, op1=Alu.add)
    rb_ctx.close()
    wv_ctx.close()

    # ---- phases 3+4 software-pipelined per head-pair ----
    attn_sb = [singles.tile([P, T], BF16, name=f"attn{c}", tag=f"attn{c}")
               for c in range(NPAIR)]

    p1ps_ctx = contextlib.ExitStack()
    p1ps = p1ps_ctx.enter_context(tc.tile_pool(name="p1psum", bufs=2,
                                               space="PSUM"))
    scps = p1ps_ctx.enter_context(tc.tile_pool(name="scps", bufs=2,
                                               space="PSUM"))
    avps = p1ps_ctx.enter_context(tc.tile_pool(name="avps", bufs=2,
                                               space="PSUM"))
    qkp = p1ps_ctx.enter_context(tc.tile_pool(name="qkp", bufs=2))
    attp = p1ps_ctx.enter_context(tc.tile_pool(name="attp", bufs=2))
    lp = p1ps_ctx.enter_context(tc.tile_pool(name="lp", bufs=3))

    def phase_qk(c):
        """Project Q_c and K_c into (j, t) bf16 tiles; returns (q, k)."""
        tiles = {}
        for which, jc in (("q", c), ("k", NQ + c)):
            qk = qkp.tile([P, T], BF16, name=f"{which}T{c}", tag=f"{which}T")
            # tf-inner so consecutive matmuls share the stationary operand
            pss = [p1ps.tile([P, TF], F32, name=f"ps{tf}", tag="ps")
                   for tf in range(NTF)]
            for dc in range(ND):
                for tf in range(NTF):
                    nc.tensor.matmul(
                        pss[tf], lhsT=wqk_sb[dc][:, jc * P:(jc + 1) * P],
                        rhs=xr_sb[dc][:, tf * TF:(tf + 1) * TF],
                        start=(dc == 0), stop=(dc == ND - 1))
            for tf in range(NTF):
                ts = slice(tf * TF, (tf + 1) * TF)
                nc.vector.tensor_scalar_add(out=qk[:, ts], in0=pss[tf],
                                            scalar1=c2qk_sb[:, jc:jc + 1])
            tiles[which] = qk
        return tiles["q"], tiles["k"]

    def phase_scores(c, qtile, ktile):
        """scores -> exp (kv-mask bias) -> causal stripe; returns atts."""
        atts = {}
        for sc in range(NT):
            w = T - sc * P  # valid columns for this key block
            for h01 in (0, 1):
                hp = slice(h01 * hd, (h01 + 1) * hd)
                att = attp.tile([P, w], BF16, name=f"att{h01}_{sc}",
                                tag=f"att{h01}_{sc}")
                atts[(h01, sc)] = att
                # both tf windows share the stationary K block: MMs
                # back-to-back, exps after.
                work = []
                for tf in range(NTF):
                    lo = max(sc * P, tf * TF)  # global first valid col
                    hi = (tf + 1) * TF
                    if lo >= hi:
                        continue
                    ps = scps.tile([P, TF], F32, name=f"scp{h01}_{tf}",
                                   tag=f"scp{h01}")
                    nc.tensor.matmul(
                        ps[:, lo - tf * TF:TF],
                        lhsT=ktile[hp, sc * P:(sc + 1) * P],
                        rhs=qtile[hp, lo:hi],
                        start=True, stop=True,
                        tile_position=(h01 * hd, 0))
                    work.append((ps, lo, hi, tf))
                for ps, lo, hi, tf in work:
                    nc.scalar.activation(
                        out=att[:, lo - sc * P:hi - sc * P],
                        in_=ps[:, lo - tf * TF:TF], func=Act.Exp,
                        bias=kvm_sb[:, sc:sc + 1], scale=scale)
            for h01 in (0, 1):
                # diagonal stripe: zero strictly-upper within the block
                nc.gpsimd.affine_select(
                    out=atts[(h01, sc)][:, 0:P],
                    in_=atts[(h01, sc)][:, 0:P],
                    pattern=[[1, P]], compare_op=Alu.is_ge, fill=0.0,
                    base=0, channel_multiplier=-1)
        return atts

    def phase_av(c, atts):
        """att @ V (+ denominator), normalize into attn_sb[c].

        tf-inner so the two psum accumulations share each stationary V
        block; pso is evacuated to SBUF immediately (frees the bank), the
        slow recip/broadcast chain then runs out of SBUF.
        """
        for h01 in (0, 1):
            h = 2 * c + h01
            psos = [avps.tile([hd + 1, TF], F32, name=f"pso{tf}", tag="pso")
                    for tf in range(NTF)]
            for sc in range(NT):
                for tf in range(NTF):
                    lo = max(sc * P, tf * TF)
                    hi = (tf + 1) * TF
                    if lo >= hi:
                        continue
                    n_sc = min(NT, (tf + 1) * TF // P)
                    nc.tensor.matmul(
                        psos[tf][:, lo - tf * TF:TF],
                        lhsT=v_pad_sb[sc][:, h, 0:hd + 1],
                        rhs=atts[(h01, sc)][:, lo - sc * P:hi - sc * P],
                        start=(sc == 0), stop=(sc == n_sc - 1))
            for tf in range(NTF):
                po = lp.tile([hd + 1, TF], F32, name=f"po{tf}", tag="po")
                nc.vector.tensor_copy(out=po, in_=psos[tf])
                nc.vector.reciprocal_approx_fast(out=po[hd:hd + 1, :],
                                                 in_=po[hd:hd + 1, :])
                lb = lp.tile([hd, TF], F32, name=f"lb{tf}", tag="lb")
                nc.gpsimd.partition_broadcast(out_ap=lb, in_ap=po[hd:hd + 1, :])
                nc.vector.tensor_mul(
                    out=attn_sb[c][h01 * hd:(h01 + 1) * hd,
                                   tf * TF:(tf + 1) * TF],
                    in0=po[0:hd, :], in1=lb)

    q0, k0 = phase_qk(0)
    prev = (0, phase_scores(0, q0, k0))
    for c in range(1, NPAIR):
        q, k = phase_qk(c)
        atts = phase_scores(c, q, k)
        phase_av(*prev)
        prev = (c, atts)
    phase_av(*prev)
    p1ps_ctx.close()
    wqk_ctx.close()

    # ---- phase 5: output projection, (e, t) layout ----
    with tc.tile_pool(name="p4psum", bufs=4, space="PSUM") as p4ps, \
         tc.tile_pool(name="p4tmp", bufs=3) as p4tmp:
        for ec in range(ND):
            pss = [p4ps.tile([P, TF], F32, name=f"ps{tf}", tag="ps")
                   for tf in range(NTF)]
            for vc in range(ND):
                for tf in range(NTF):
                    nc.tensor.matmul(
                        pss[tf], lhsT=wout_sb[vc][:, ec * P:(ec + 1) * P],
                        rhs=attn_sb[vc][:, tf * TF:(tf + 1) * TF],
                        start=(vc == 0), stop=(vc == ND - 1))
            for tf in range(NTF):
                ts = slice(tf * TF, (tf + 1) * TF)
                ot = p4tmp.tile([P, TF], BF16, name=f"ot{tf}", tag="ot")
                nc.scalar.activation(out=ot, in_=pss[tf], func=Act.Identity,
                                     bias=bout_sb[:, ec:ec + 1])
                nc.sync.dma_start(out=out[ec * P:(ec + 1) * P, ts], in_=ot)
    ctx.close()


def host_inputs(xb, x_len, gamma, beta, w_qkv, b_qkv, w_out, b_out,
                T=1024, D=1024, H=16):
    """Build the per-core input map (numpy) for the bass program."""
    bf16 = ml_dtypes.bfloat16
    ND = D // P
    NT = T // P
    NJQK = 2 * D // P

    Wp = (gamma[:, None] * w_qkv).astype(np.float32)
    c1 = Wp.sum(0)
    c2 = (beta @ w_qkv + b_qkv).astype(np.float32)

    xT_bf = np.ascontiguousarray(xb.T).astype(bf16)
    wqk_bf = np.ascontiguousarray(Wp[:, :2 * D]).astype(bf16)
    wv_bf = np.ascontiguousarray(Wp[:, 2 * D:]).astype(bf16)
    wout_bf = np.ascontiguousarray(w_out).astype(bf16)

    c2qk = np.ascontiguousarray(c2[:2 * D].reshape(NJQK, P).T).astype(np.float32)
    c1v = np.ascontiguousarray(c1[2 * D:].reshape(1, D)).astype(np.float32)
    bv = np.ascontiguousarray(c2[2 * D:].reshape(1, D)).astype(np.float32)

    kv = np.where(np.arange(T) < int(x_len), 0.0, NEG).astype(np.float32)
    kvm = np.ascontiguousarray(kv.reshape(NT, P).T).astype(np.float32)

    bo = np.ascontiguousarray(b_out.reshape(ND, P).T).astype(np.float32)

    return {
        "xT": xT_bf,
        "wqk": wqk_bf, "wv": wv_bf, "wout": wout_bf,
        "c2qk": c2qk, "c1v": c1v, "bv": bv,
        "kvm": kvm, "bout": bo,
    }


_COMPILED = {}


def _get_program():
    key = (T_FULL, D_FULL, H_FULL)
    if key not in _COMPILED:
        import concourse.tile as tile
        from concourse import bacc
        nc = bacc.Bacc("TRN2", target_bir_lowering=False, debug=False,
                       num_devices=B)
        with tile.TileContext(nc) as tc:
            build_attention(nc, tc, T=T_FULL, D=D_FULL, H=H_FULL, EPS=EPS)
        nc.compile()
        _COMPILED[key] = nc
    return _COMPILED[key]


def _run(inputs, trace=False):
    from concourse.bass_utils import run_bass_kernel_spmd

    x = np.asarray(inputs["x"], np.float32)
    x_lens = np.asarray(inputs["x_lens"])
    gamma = np.asarray(inputs["ln_gamma"], np.float32)
    beta = np.asarray(inputs["ln_beta"], np.float32)
    w_qkv = np.asarray(inputs["w_qkv"], np.float32)
    b_qkv = np.asarray(inputs["b_qkv"], np.float32)
    w_out = np.asarray(inputs["w_out"], np.float32)
    b_out = np.asarray(inputs["b_out"], np.float32)

    nc = _get_program()
    in_maps = [
        host_inputs(x[b], int(x_lens[b]), gamma, beta, w_qkv, b_qkv,
                    w_out, b_out, T=T_FULL, D=D_FULL, H=H_FULL)
        for b in range(B)
    ]
    res = run_bass_kernel_spmd(nc, in_maps, list(range(B)), trace=trace)
    out = np.stack([np.asarray(res.results[b]["out"]).astype(np.float32).T
                    for b in range(B)])
    return out, res


def kernel(**inputs):
    out, _ = _run(inputs, trace=False)
    return out


def kernel_traced(**inputs):
    """Like kernel() but also returns the SPMD run results (exec_time_ns...)."""
    import types
    try:
        from trn_agent_boot.trn_boot import _ntff_profile_via_ctypes
        hook = _ntff_profile_via_ctypes('/opt/axon/libaxon_pjrt.so')
        m = types.ModuleType('antenv.axon_hooks')
        m.get_axon_ntff_profile_hook = lambda: hook
        sys.modules.setdefault('antenv.axon_hooks', m)
    except Exception:
        pass
    out, res = _run(inputs, trace=True)
    return out, res
